# revision 3
# baseline (speedup 1.0000x reference)
"""Trainium2 Bass kernel for the GraphicalBranch GNN message-passing problem.

Math (equivalent to the reference):
  - Fully-connected edges with self-loops => segment_sum == per-sample row-sum
    S[b] broadcast over the 28 pair-nodes.
  - The final gather commutes with the linear layer, so W_self runs only on
    the 10 gathered rows/sample:
        out[b*10+k] = relu(xg[b*10+k] @ W_self + (S[b] @ W_nbr + b))
  - rows computed on host from slicing_tensor/object_pairs (index arithmetic
    identical to the reference LUT).

Sharding: data-parallel, 128 samples/core (3584 x-rows, 1280 out rows),
weights replicated.

v2 changes vs the 51us baseline (trace-driven):
  - x and xgT travel as int8 (x*32 rounded; exactly representable in bf16
    after the SWDGE cast-DMA) with the 1/32 scale folded into W_self/W_nbr
    on host.  Halves the dominant HBM-read traffic.  Sum of int8 values in
    fp32 PSUM is exact; host-simulated end-to-end rel err 1.0e-2 < 2e-2.
  - Output stored bf16 (host upcasts to f32): halves store traffic.
  - Three DMA queues: sync ring carries the small bf16 tensors, the SWDGE
    (gpsimd) ring carries the casting int8 loads, scalar ring the stores.
  - PE order: warm-up matmuls (HAM un-throttle) -> W_self tiles 0-5 while
    xgT/x stream -> S-matmuls round-robin over the 4 PSUM col-groups (the
    4 one-hot chunk groups stream concurrently through separate XBUSes)
    -> transpose/A -> expansion closes interleaved with W_self tiles 6-9.
  - ReLU alternates ScalarE/VectorE; stores issue per tile pair.
"""

import numpy as np
import ml_dtypes

# ---- problem constants (hardcoded; kernel.py must be self-contained) ----
B = 1024          # samples
NOBJ = 8          # objects per sample
NC2 = 28          # pair-nodes per sample
MAXR = 10         # relations per sample
D = 512           # feature dim
NCORES = 8
BL = B // NCORES          # 128 samples per core
RL = BL * NC2             # 3584 x-rows per core
ML = BL * MAXR            # 1280 output rows per core
KT = D // 128             # 4 contraction tiles
MT = ML // 128            # 10 output row tiles per core
XCH = 4                   # x chunks (896 rows = 32 samples each)
RJ = (RL // 128) // XCH   # 7 row-tiles per chunk
SW = BL // XCH            # 32 samples per chunk
XSC = 32.0                # int8 quantization scale for x (folded into W)
N_WARM = 20               # PE warm-up matmuls

BF16 = ml_dtypes.bfloat16

_compiled = None


def _build_bass():
    import concourse.bacc as bacc
    import concourse.bass as bass
    import concourse.mybir as mybir
    from concourse import tile

    f32 = mybir.dt.float32
    bf16 = mybir.dt.bfloat16
    i8 = mybir.dt.int8

    nc = bacc.Bacc("TRN2", target_bir_lowering=False, debug=False,
                   num_devices=NCORES)

    x_d = nc.dram_tensor("x", [XCH, 128, RJ * D], i8, kind="ExternalInput")
    xgT_d = nc.dram_tensor("xgT", [128, KT * ML], i8, kind="ExternalInput")
    ws_d = nc.dram_tensor("ws", [128, KT * D], bf16, kind="ExternalInput")
    wn_d = nc.dram_tensor("wn", [128, KT * D], bf16, kind="ExternalInput")
    g_d = nc.dram_tensor("g", [128, RJ * SW], bf16, kind="ExternalInput")
    eT_d = nc.dram_tensor("eT", [128, ML], bf16, kind="ExternalInput")
    b_d = nc.dram_tensor("bias", [1, D], bf16, kind="ExternalInput")
    id_d = nc.dram_tensor("ident", [128, 128], bf16, kind="ExternalInput")
    out_d = nc.dram_tensor("out", [ML, D], bf16, kind="ExternalOutput")

    with tile.TileContext(nc) as tc:
        with (
            tc.tile_pool(name="const", bufs=1) as cpool,
            tc.tile_pool(name="x", bufs=XCH) as xpool,
            tc.tile_pool(name="outp", bufs=3) as opool,
            tc.tile_pool(name="psum", bufs=6, space=bass.MemorySpace.PSUM) as ppool,
            tc.tile_pool(name="psumS", bufs=1, space=bass.MemorySpace.PSUM) as pspool,
            tc.tile_pool(name="psumT", bufs=1, space=bass.MemorySpace.PSUM) as ptpool,
        ):
            # ---- small on-chip constants ----
            warm_sb = cpool.tile([128, 128], bf16)
            nc.gpsimd.memset(warm_sb[:], 1.0)
            ones_sb = cpool.tile([1, 128], bf16)
            nc.gpsimd.memset(ones_sb[:], 1.0)

            # ---- loads: sync ring = small bf16 set, SWDGE ring = int8 casts
            ws_sb = cpool.tile([128, KT, D], bf16)
            nc.sync.dma_start(ws_sb[:], ws_d.rearrange("p (t n) -> p t n", n=D))
            id_sb = cpool.tile([128, 128], bf16)
            nc.sync.dma_start(id_sb[:], id_d[:, :])
            g_sb = cpool.tile([128, RJ, SW], bf16)
            nc.sync.dma_start(g_sb[:], g_d.rearrange("p (j s) -> p j s", s=SW))
            wn_sb = cpool.tile([128, KT, D], bf16)
            nc.sync.dma_start(wn_sb[:], wn_d.rearrange("p (t n) -> p t n", n=D))
            b_sb = cpool.tile([1, D], bf16)
            nc.sync.dma_start(b_sb[:], b_d[:, :])
            eT_sb = cpool.tile([128, ML], bf16)
            nc.sync.dma_start(eT_sb[:], eT_d[:, :])

            xgT_sb = cpool.tile([128, KT, ML], bf16)
            nc.gpsimd.dma_start(xgT_sb[:],
                                xgT_d.rearrange("p (t m) -> p t m", m=ML))
            x_sb = []
            for ch in range(XCH):
                xch = xpool.tile([128, RJ, D], bf16, tag="x")
                nc.gpsimd.dma_start(xch[:],
                                    x_d[ch].rearrange("p (j d) -> p j d", d=D))
                x_sb.append(xch)

            # ---- PE warm-up: garbage matmuls to lift the HAM throttle ----
            warm_ps = ptpool.tile([128, 128], f32, tag="warm")
            for i in range(N_WARM):
                nc.tensor.matmul(warm_ps[:], warm_sb[:], warm_sb[:],
                                 start=(i == 0), stop=(i == N_WARM - 1))

            # ---- W_self accumulation groups (tiles close later via E@a) --
            main_ps = {}

            def open_main_group(t):
                ps = ppool.tile([128, D], f32, tag="ps")
                for kt in range(KT):
                    nc.tensor.matmul(
                        ps[:],
                        xgT_sb[:, kt, t * 128:(t + 1) * 128],
                        ws_sb[:, kt, :],
                        start=(kt == 0), stop=False,
                    )
                main_ps[t] = ps

            for t in range(6):
                open_main_group(t)

            # ---- S: round-robin the 4 col-groups so their chunk streams
            # ---- overlap on separate XBUSes ----
            psS = pspool.tile([128, D], f32, tag="pS")
            for j in range(RJ):
                for ch in range(XCH):
                    nc.tensor.matmul(psS[ch * SW:(ch + 1) * SW, :],
                                     g_sb[:, j, :], x_sb[ch][:, j, :],
                                     start=(j == 0), stop=(j == RJ - 1),
                                     tile_position=(0, ch * SW))

            # ---- S -> S^T (bf16) via PE transposes; copies split over
            # ---- scalar+vector ----
            s_nat = cpool.tile([128, D], bf16)
            s_bf = cpool.tile([128, KT, BL], bf16)
            psT = pspool.tile([128, KT * 128], bf16, tag="pS")
            for dt in range(KT):
                nc.scalar.copy(s_nat[:, dt * 128:(dt + 1) * 128],
                               psS[:, dt * 128:(dt + 1) * 128])
                nc.tensor.transpose(psT[:, dt * 128:(dt + 1) * 128],
                                    s_nat[:, dt * 128:(dt + 1) * 128],
                                    id_sb[:])
            for dt in range(KT):
                eng = nc.vector.tensor_copy if dt % 2 == 0 else nc.scalar.copy
                eng(s_bf[:, dt, :], psT[:, dt * 128:(dt + 1) * 128])

            # ---- A = S @ W_nbr + b (bias via K=1 ones matmul) ----
            psA = pspool.tile([128, D], f32, tag="pS")
            for kt in range(KT):
                nc.tensor.matmul(psA[:], s_bf[:, kt, :], wn_sb[:, kt, :],
                                 start=(kt == 0), stop=False)
            nc.tensor.matmul(psA[:], ones_sb[:], b_sb[:],
                             start=False, stop=True)
            a_bf = cpool.tile([128, D], bf16)
            nc.vector.tensor_copy(a_bf[:], psA[:])

            # ---- close groups (expansion adds broadcast A), interleaved
            # ---- with the remaining W_self tiles; ReLU alternates engines;
            # ---- stores per tile pair on the scalar ring ----
            out_r = out_d.rearrange("(t u p) n -> t p u n", p=128, u=2)
            relu = mybir.ActivationFunctionType.Relu
            ot = None
            next_open = 6
            for t in range(MT):
                ps = main_ps.pop(t)
                nc.tensor.matmul(ps[:], eT_sb[:, t * 128:(t + 1) * 128],
                                 a_bf[:], start=False, stop=True)
                if t % 2 == 0:
                    ot = opool.tile([128, 2, D], bf16, tag="ot")
                if t % 2 == 0:
                    nc.scalar.activation(ot[:, 0, :], ps[:], relu)
                else:
                    nc.vector.tensor_scalar_max(ot[:, 1, :], ps[:], 0.0)
                if next_open < MT:
                    open_main_group(next_open)
                    next_open += 1
                if t % 2 == 1:
                    nc.scalar.dma_start(out_r[t // 2], ot[:])

    nc.compile()
    return nc


def _get_compiled():
    global _compiled
    if _compiled is None:
        _compiled = _build_bass()
    return _compiled


def _host_prep(inputs):
    """Shard + preprocess on host. Returns per-core input maps."""
    x = np.asarray(inputs["spatial_branch_feature_map"], dtype=np.float32)
    W_self = np.asarray(inputs["W_self"], dtype=np.float32)
    W_nbr = np.asarray(inputs["W_nbr"], dtype=np.float32)
    b = np.asarray(inputs["b"], dtype=np.float32)
    st = np.asarray(inputs["slicing_tensor"])
    op = np.asarray(inputs["object_pairs"])

    N = x.shape[0]
    n = NOBJ
    # exact replication of the reference's LUT-based row computation
    keys = st[:, 0].astype(np.int64) * (n * n) + st[:, 1].astype(np.int64) * n \
        + st[:, 2].astype(np.int64)
    lut = np.zeros(B * n * n, dtype=np.int64)
    lut[keys] = np.arange(N, dtype=np.int64)
    pmin = np.minimum(op[..., 0], op[..., 1]).astype(np.int64)
    pmax = np.maximum(op[..., 0], op[..., 1]).astype(np.int64)
    rel_keys = (np.arange(B, dtype=np.int64)[:, None] * (n * n)
                + pmin * n + pmax).reshape(-1)
    rows = lut[rel_keys]                      # [B*MAXR] global row index

    xq = np.clip(np.round(x * XSC), -127, 127).astype(np.int8)
    xgq = xq[rows]                            # [B*MAXR, D] int8
    # x: [NCORES, XCH, 128, RJ*D]; sbuf[p, j, :] = x_core[ch*896 + j*128 + p]
    x_i8 = np.ascontiguousarray(
        xq.reshape(NCORES, XCH, RJ, 128, D)
        .transpose(0, 1, 3, 2, 4).reshape(NCORES, XCH, 128, RJ * D))
    # xgT: [NCORES, 128, KT*ML]; sbuf[p, kt, m] = xg_core[m, kt*128+p]
    xgT = np.ascontiguousarray(
        xgq.reshape(NCORES, ML, KT, 128)
        .transpose(0, 3, 2, 1).reshape(NCORES, 128, KT * ML))

    def wlay(W):  # [D, D] -> [128, KT*D]: sbuf[p, kt, n] = W[kt*128+p, n]
        return np.ascontiguousarray(
            (W / np.float32(XSC)).astype(BF16).reshape(KT, 128, D)
            .transpose(1, 0, 2).reshape(128, KT * D))

    ws = wlay(W_self)
    wn = wlay(W_nbr)
    eT = (np.arange(ML)[None, :] // MAXR
          == np.arange(128)[:, None]).astype(BF16)   # [128, ML]
    # shared one-hot block: g[p, j*SW + s] = ((j*128 + p)//NC2 == s)
    jj = np.arange(RJ * 128)
    g = (jj[:, None] // NC2 == np.arange(SW)[None, :]).astype(BF16)
    g = np.ascontiguousarray(
        g.reshape(RJ, 128, SW).transpose(1, 0, 2).reshape(128, RJ * SW))
    bias = b.astype(BF16).reshape(1, D)
    ident = np.eye(128, dtype=BF16)

    in_maps = []
    for c in range(NCORES):
        in_maps.append({
            "x": x_i8[c], "xgT": xgT[c], "g": g,
            "ws": ws, "wn": wn, "eT": eT, "bias": bias, "ident": ident,
        })
    return in_maps


def run(inputs, trace=False):
    """Returns (full_output, BassKernelResults)."""
    from concourse.bass_utils import run_bass_kernel_spmd

    nc = _get_compiled()
    in_maps = _host_prep(inputs)
    res = run_bass_kernel_spmd(nc, in_maps, core_ids=list(range(NCORES)),
                               trace=trace)
    out = np.concatenate([r["out"] for r in res.results],
                         axis=0).astype(np.float32)
    return out, res


def kernel(**inputs) -> np.ndarray:
    out, _ = run(inputs, trace=False)
    return out


# revision 9
# speedup vs baseline: 1.0001x; 1.0001x over previous
"""Trainium2 Bass kernel for the GraphicalBranch GNN message-passing problem.

Math (equivalent to the reference):
  - Fully-connected edges with self-loops => segment_sum == per-sample row-sum
    S[b] broadcast over the 28 pair-nodes.
  - The final gather commutes with the linear layer, so W_self runs only on
    the 10 gathered rows/sample:
        out[b*10+k] = relu(xg[b*10+k] @ W_self + (S[b] @ W_nbr + b))
  - rows computed on host from slicing_tensor/object_pairs (index arithmetic
    identical to the reference LUT).

Sharding: data-parallel, 128 samples/core (3584 x-rows, 1280 out rows),
weights replicated.

v3 schedule (trace-driven):
  - All loads bf16 over the two HWDGE rings + one plain SWDGE ring; output
    stored bf16 (host upcasts).  Total HBM traffic 7.7MB/core.
  - ws + the first xgT slices get the whole HBM pipe first: the x-chunk
    DMAs sit behind a tiny ws-dependent dummy store in their FIFO queues,
    so W_self starts ~4us earlier.
  - xgT arrives in 5 tile-pair slices (progressive W_self); x chunks are
    interleaved across the scalar/gpsimd queues so samples 0-63 land first.
  - S-matmuls run in 2-chunk round-robin pairs: the two PSUM col-groups
    stream concurrently through separate XBUSes (~2x).
  - A/transposes/expansion run per 64-sample half so output tiles close,
    ReLU (alternating ScalarE/VectorE) and store progressively.
  - Warm-up matmuls keep the PE HAM un-throttled through the initial wait.
"""

import numpy as np
import ml_dtypes

# ---- problem constants (hardcoded; kernel.py must be self-contained) ----
B = 1024          # samples
NOBJ = 8          # objects per sample
NC2 = 28          # pair-nodes per sample
MAXR = 10         # relations per sample
D = 512           # feature dim
NCORES = 8
BL = B // NCORES          # 128 samples per core
RL = BL * NC2             # 3584 x-rows per core
ML = BL * MAXR            # 1280 output rows per core
KT = D // 128             # 4 contraction tiles
MT = ML // 128            # 10 output row tiles per core
XCH = 4                   # x chunks (896 rows = 32 samples each)
RJ = (RL // 128) // XCH   # 7 row-tiles per chunk
SW = BL // XCH            # 32 samples per chunk
N_WARM = 40               # PE warm-up matmuls (N=128, fill the DMA wait)

BF16 = ml_dtypes.bfloat16

_compiled = None


def _build_bass():
    import concourse.bacc as bacc
    import concourse.bass as bass
    import concourse.mybir as mybir
    from concourse import tile

    f32 = mybir.dt.float32
    bf16 = mybir.dt.bfloat16

    nc = bacc.Bacc("TRN2", target_bir_lowering=False, debug=False,
                   num_devices=NCORES)

    x_d = nc.dram_tensor("x", [XCH, 128, RJ * D], bf16, kind="ExternalInput")
    xgT_d = nc.dram_tensor("xgT", [128, MT * KT * 128], bf16,
                           kind="ExternalInput")
    ws_d = nc.dram_tensor("ws", [128, KT * D], bf16, kind="ExternalInput")
    wn_d = nc.dram_tensor("wn", [128, KT * D], bf16, kind="ExternalInput")
    g_d = nc.dram_tensor("g", [128, RJ * SW], bf16, kind="ExternalInput")
    eT_d = nc.dram_tensor("eT", [128, ML], bf16, kind="ExternalInput")
    b_d = nc.dram_tensor("bias", [1, D], bf16, kind="ExternalInput")
    id_d = nc.dram_tensor("ident", [128, 128], bf16, kind="ExternalInput")
    gate_d = nc.dram_tensor("gate", [1, 32], bf16, kind="ExternalOutput")
    out_d = nc.dram_tensor("out", [ML, D], bf16, kind="ExternalOutput")

    with tile.TileContext(nc) as tc:
        with (
            tc.tile_pool(name="const", bufs=1) as cpool,
            tc.tile_pool(name="x", bufs=XCH) as xpool,
            tc.tile_pool(name="outp", bufs=3) as opool,
            tc.tile_pool(name="psum", bufs=5, space=bass.MemorySpace.PSUM) as ppool,
            tc.tile_pool(name="psumS", bufs=1, space=bass.MemorySpace.PSUM) as pspool,
            tc.tile_pool(name="psumA", bufs=1, space=bass.MemorySpace.PSUM) as papool,
            tc.tile_pool(name="psumT", bufs=1, space=bass.MemorySpace.PSUM) as ptpool,
        ):
            warm_sb = cpool.tile([128, 128], bf16)
            nc.gpsimd.memset(warm_sb[:], 1.0)
            ones_sb = cpool.tile([1, 128], bf16)
            nc.gpsimd.memset(ones_sb[:], 1.0)

            # ---- sync ring: ws first, then xgT tile-pair slices etc. ----
            ws_sb = cpool.tile([128, KT, D], bf16)
            nc.sync.dma_start(ws_sb[:], ws_d.rearrange("p (t n) -> p t n", n=D))
            xgT_sb = cpool.tile([128, MT, KT, 128], bf16)
            xgT_r = xgT_d.rearrange("p (t k m) -> p t k m", k=KT, m=128)
            for a, b_ in ((0, 2), (2, 4)):
                nc.sync.dma_start(xgT_sb[:, a:b_], xgT_r[:, a:b_])
            wn_sb = cpool.tile([128, KT, D], bf16)
            nc.sync.dma_start(wn_sb[:], wn_d.rearrange("p (t n) -> p t n", n=D))
            id_sb = cpool.tile([128, 128], bf16)
            nc.sync.dma_start(id_sb[:], id_d[:, :])
            b_sb = cpool.tile([1, D], bf16)
            nc.sync.dma_start(b_sb[:], b_d[:, :])
            eT_sb = cpool.tile([128, ML], bf16)
            nc.sync.dma_start(eT_sb[:], eT_d[:, :])
            for a, b_ in ((4, 6), (6, 8), (8, 10)):
                nc.sync.dma_start(xgT_sb[:, a:b_], xgT_r[:, a:b_])

            # ---- scalar ring: g, then (gated) x chunks 0 and 2 ----
            g_sb = cpool.tile([128, RJ, SW], bf16)
            nc.scalar.dma_start(g_sb[:], g_d.rearrange("p (j s) -> p j s", s=SW))
            # FIFO gate: this store depends on ws, so the x transfers behind
            # it cannot start before ws has fully landed.
            nc.scalar.dma_start(gate_d[:, 0:16], ws_sb[0:1, 0, 0:16])
            x_sb = [None] * XCH
            for ch in (0, 2):
                xch = xpool.tile([128, RJ, D], bf16, tag="x")
                nc.scalar.dma_start(xch[:],
                                    x_d[ch].rearrange("p (j d) -> p j d", d=D))
                x_sb[ch] = xch

            # ---- gpsimd ring: (gated) x chunks 1 and 3 ----
            nc.gpsimd.dma_start(gate_d[:, 16:32], ws_sb[0:1, 0, 16:32])
            for ch in (1, 3):
                xch = xpool.tile([128, RJ, D], bf16, tag="x")
                nc.gpsimd.dma_start(xch[:],
                                    x_d[ch].rearrange("p (j d) -> p j d", d=D))
                x_sb[ch] = xch

            # ---- PE warm-up: keep HAM un-throttled through the DMA wait --
            warm_ps = ptpool.tile([128, 128], f32, tag="pT")
            for i in range(N_WARM):
                nc.tensor.matmul(warm_ps[:], warm_sb[:], warm_sb[:],
                                 start=(i == 0), stop=(i == N_WARM - 1))

            main_ps = {}

            def open_main_group(t):
                ps = ppool.tile([128, D], f32, tag="ps")
                for kt in range(KT):
                    nc.tensor.matmul(
                        ps[:],
                        xgT_sb[:, t, kt, :],
                        ws_sb[:, kt, :],
                        start=(kt == 0), stop=False,
                    )
                main_ps[t] = ps

            psS = pspool.tile([128, D], f32)
            psA = papool.tile([128, D], f32)
            s_nat = cpool.tile([128, D], bf16)
            s_bf = cpool.tile([128, KT, BL], bf16)
            a_bf = cpool.tile([128, D], bf16)
            psT = ptpool.tile([128, KT, 2, 64], bf16, tag="pT")

            def s_pair(c0, c1):
                # two chunks' one-hot matmuls round-robin over their two
                # PSUM col-groups -> concurrent XBUS streams
                for j in range(RJ):
                    for ch in (c0, c1):
                        nc.tensor.matmul(psS[ch * SW:(ch + 1) * SW, :],
                                         g_sb[:, j, :], x_sb[ch][:, j, :],
                                         start=(j == 0), stop=(j == RJ - 1),
                                         tile_position=(0, ch * SW))

            def half_A(h):
                lo, hi = h * 64, (h + 1) * 64
                for dt in range(KT):
                    eng = nc.scalar.copy if dt % 2 == 0 else nc.vector.tensor_copy
                    eng(s_nat[lo:hi, dt * 128:(dt + 1) * 128],
                        psS[lo:hi, dt * 128:(dt + 1) * 128])
                    nc.tensor.transpose(psT[:, dt, h, :],
                                        s_nat[lo:hi, dt * 128:(dt + 1) * 128],
                                        id_sb[lo:hi, lo:hi])
                for dt in range(KT):
                    nc.vector.tensor_copy(s_bf[:, dt, lo:hi], psT[:, dt, h, :])
                for kt in range(KT):
                    nc.tensor.matmul(psA[lo:hi, :], s_bf[:, kt, lo:hi],
                                     wn_sb[:, kt, :],
                                     start=(kt == 0), stop=False,
                                     tile_position=(0, lo))
                nc.tensor.matmul(psA[lo:hi, :], ones_sb[:, 0:64], b_sb[:],
                                 start=False, stop=True,
                                 tile_position=(0, lo))
                nc.scalar.copy(a_bf[lo:hi, :], psA[lo:hi, :])

            out_r = out_d.rearrange("(t u p) n -> t p u n", p=128, u=2)
            relu = mybir.ActivationFunctionType.Relu
            ot_tiles = {}

            def close_tile(t):
                # expansion matmul broadcasts A over the tile's rows, then
                # ReLU (alternating engines) into the store buffer
                h = 0 if t < 5 else 1
                lo, hi = h * 64, (h + 1) * 64
                ps = main_ps.pop(t)
                nc.tensor.matmul(ps[:], eT_sb[lo:hi, t * 128:(t + 1) * 128],
                                 a_bf[lo:hi, :], start=False, stop=True)
                if t % 2 == 0:
                    ot_tiles[t // 2] = opool.tile([128, 2, D], bf16, tag="ot",
                                                  name=f"ot{t // 2}")
                ot = ot_tiles[t // 2]
                if t % 2 == 0:
                    nc.scalar.activation(ot[:, 0, :], ps[:], relu)
                else:
                    nc.vector.tensor_scalar_max(ot[:, 1, :], ps[:], 0.0)
                    nc.sync.dma_start(out_r[t // 2], ot[:])

            # ---- PE program ----
            open_main_group(0)
            open_main_group(1)
            open_main_group(2)
            open_main_group(3)
            s_pair(0, 1)
            half_A(0)
            close_tile(0)
            close_tile(1)
            close_tile(2)
            close_tile(3)
            open_main_group(4)
            open_main_group(5)
            close_tile(4)
            s_pair(2, 3)
            half_A(1)
            close_tile(5)
            open_main_group(6)
            open_main_group(7)
            close_tile(6)
            close_tile(7)
            open_main_group(8)
            open_main_group(9)
            close_tile(8)
            close_tile(9)

    nc.compile()
    return nc


def _get_compiled():
    global _compiled
    if _compiled is None:
        _compiled = _build_bass()
    return _compiled


def _host_prep(inputs):
    """Shard + preprocess on host. Returns per-core input maps."""
    x = np.asarray(inputs["spatial_branch_feature_map"], dtype=np.float32)
    W_self = np.asarray(inputs["W_self"], dtype=np.float32)
    W_nbr = np.asarray(inputs["W_nbr"], dtype=np.float32)
    b = np.asarray(inputs["b"], dtype=np.float32)
    st = np.asarray(inputs["slicing_tensor"])
    op = np.asarray(inputs["object_pairs"])

    N = x.shape[0]
    n = NOBJ
    # exact replication of the reference's LUT-based row computation
    keys = st[:, 0].astype(np.int64) * (n * n) + st[:, 1].astype(np.int64) * n \
        + st[:, 2].astype(np.int64)
    lut = np.zeros(B * n * n, dtype=np.int64)
    lut[keys] = np.arange(N, dtype=np.int64)
    pmin = np.minimum(op[..., 0], op[..., 1]).astype(np.int64)
    pmax = np.maximum(op[..., 0], op[..., 1]).astype(np.int64)
    rel_keys = (np.arange(B, dtype=np.int64)[:, None] * (n * n)
                + pmin * n + pmax).reshape(-1)
    rows = lut[rel_keys]                      # [B*MAXR] global row index

    xg = x[rows]                              # [B*MAXR, D]
    # x: [NCORES, XCH, 128, RJ*D]; sbuf[p, j, :] = x_core[ch*896 + j*128 + p]
    x_bf = np.ascontiguousarray(
        x.astype(BF16).reshape(NCORES, XCH, RJ, 128, D)
        .transpose(0, 1, 3, 2, 4).reshape(NCORES, XCH, 128, RJ * D))
    # xgT: [NCORES, 128, MT*KT*128]; sbuf[p, t, kt, m] = xg_core[t*128+m, kt*128+p]
    xgT = np.ascontiguousarray(
        xg.astype(BF16).reshape(NCORES, MT, 128, KT, 128)
        .transpose(0, 4, 1, 3, 2).reshape(NCORES, 128, MT * KT * 128))

    def wlay(W):  # [D, D] -> [128, KT*D]: sbuf[p, kt, n] = W[kt*128+p, n]
        return np.ascontiguousarray(
            W.astype(BF16).reshape(KT, 128, D).transpose(1, 0, 2)
            .reshape(128, KT * D))

    ws = wlay(W_self)
    wn = wlay(W_nbr)
    eT = (np.arange(ML)[None, :] // MAXR
          == np.arange(128)[:, None]).astype(BF16)   # [128, ML]
    # shared one-hot block: g[p, j*SW + s] = ((j*128 + p)//NC2 == s)
    jj = np.arange(RJ * 128)
    g = (jj[:, None] // NC2 == np.arange(SW)[None, :]).astype(BF16)
    g = np.ascontiguousarray(
        g.reshape(RJ, 128, SW).transpose(1, 0, 2).reshape(128, RJ * SW))
    bias = b.astype(BF16).reshape(1, D)
    ident = np.eye(128, dtype=BF16)

    in_maps = []
    for c in range(NCORES):
        in_maps.append({
            "x": x_bf[c], "xgT": xgT[c], "g": g,
            "ws": ws, "wn": wn, "eT": eT, "bias": bias, "ident": ident,
        })
    return in_maps


def run(inputs, trace=False):
    """Returns (full_output, BassKernelResults)."""
    from concourse.bass_utils import run_bass_kernel_spmd

    nc = _get_compiled()
    in_maps = _host_prep(inputs)
    res = run_bass_kernel_spmd(nc, in_maps, core_ids=list(range(NCORES)),
                               trace=trace)
    out = np.concatenate([r["out"] for r in res.results],
                         axis=0).astype(np.float32)
    return out, res


def kernel(**inputs) -> np.ndarray:
    out, _ = run(inputs, trace=False)
    return out
